# revision 28
# baseline (speedup 1.0000x reference)
"""Multi-headed attention (RoPE + position_bias + residual attention) on 8 TRN2 cores.

Contract: kernel(**inputs) takes the FULL inputs of reference.setup_inputs()
and returns (out, prev_attn_out) as the reference does.

Sharding: 8 cores = 4 batches x 2 head-groups (8 heads each).
  - Each core computes its batch's QKV projections for its 8 heads,
    RoPE, scores (+bias/sqrt(D) + prev_attn + mask), softmax, PV, and a
    partial output projection (row-parallel over heads).
  - Host sums the two partial projections per batch and adds bo.

All device compute is emitted once as a single SPMD Bass/Tile program; the
per-core differences are entirely in the input tensors.
"""

import math
import os

import ml_dtypes
import numpy as np

def _pv_cast(a):
    return a.astype(ml_dtypes.bfloat16) if os.environ.get("KBF", "0") == "1" else a

import concourse.bacc as bacc
import concourse.bass as bass
import concourse.mybir as mybir
import concourse.tile as tile
from concourse.masks import make_identity

F32 = mybir.dt.float32
BF16 = mybir.dt.bfloat16
AF = mybir.ActivationFunctionType

B, S, HID, NH, D = 4, 1024, 2048, 16, 128
NHC = 8            # heads per core
P = 128            # partitions
NT = S // P        # 8 s-tiles
KT = HID // P      # 16 hid k-tiles
SQD = math.sqrt(float(D))

# matmul compute dtype: float32r = fp32 data, reduced-precision (fast) PE path.
# walrus requires every producer of an fp32r-matmul operand to emit float32r,
# so the operand tiles (and the DRAM tensors DMA'd straight into them) are
# declared float32r rather than bitcast at use.
MM_DT = mybir.dt.float32r if os.environ.get("KMM", "f32r") == "f32r" else F32
# optional bf16 on the probs/v/out chains (adds ~4e-3 rel err on `out`)
USE_BF = os.environ.get("KBF", "0") == "1"
PV_DT = BF16 if USE_BF else MM_DT


def emit(nc, tc, t):
    """Emit the per-core program. `t` maps tensor names -> DRAM handles."""
    import contextlib

    ctx = contextlib.ExitStack()
    with ctx:
        const = ctx.enter_context(tc.tile_pool(name="const", bufs=1))
        identity = const.tile([P, P], F32, name="identity")
        make_identity(nc, identity)
        # f32r constants cannot be produced by memset/affine_select
        # (walrus rejects the ISA) so they arrive as tiny DMA'd inputs.
        ones_row = const.tile([1, P], MM_DT, name="ones_row")
        nc.sync.dma_start(out=ones_row[:], in_=t["ones_c"][:])
        if USE_BF:
            identity_pv = const.tile([P, P], BF16, name="identity_pv")
            make_identity(nc, identity_pv)
            ones_pv = const.tile([1, P], BF16, name="ones_pv")
            nc.vector.memset(ones_pv[:], 1.0)
        else:
            identity_pv = const.tile([P, P], MM_DT, name="identity_pv")
            nc.sync.dma_start(out=identity_pv[:], in_=t["ident_c"][:])
            ones_pv = ones_row
        # identity in matmul dtype for the PE-side additive-term matmuls
        identity_mm = const.tile([P, P], MM_DT, name="identity_mm")
        nc.sync.dma_start(out=identity_mm[:], in_=t["ident_c"][:])

        # biases as [1, 1024] rows in sbuf
        brow = {}
        for nm in ("bq", "bk", "bv"):
            bdt = PV_DT if nm == "bv" else MM_DT
            brow[nm] = const.tile([1, NHC * D], bdt, name=f"{nm}_sb")
            nc.sync.dma_start(out=brow[nm][:], in_=t[nm][:])

        # persistent results of the projection phase
        qkt_pool = ctx.enter_context(tc.tile_pool(name="qkt", bufs=2 * NHC))
        QT = [qkt_pool.tile([P, S], MM_DT, name=f"QT{h}", tag="qkt") for h in range(NHC)]
        KTt = [qkt_pool.tile([P, S], MM_DT, name=f"KT{h}", tag="qkt") for h in range(NHC)]

        v_scratch = t["v_scratch"]

        # ---------------- Phase T+P: transposes + projections -----------------
        with (
            tc.tile_pool(name="xt", bufs=38) as xt_pool,
            tc.tile_pool(name="wst", bufs=KT) as w_pool,
            tc.tile_pool(name="trig", bufs=2) as trig_pool,
            tc.tile_pool(name="ropetmp", bufs=6) as rtmp_pool,
            tc.tile_pool(name="rotout", bufs=2) as rot_pool,
            tc.tile_pool(name="vout", bufs=2) as vout_pool,
            tc.tile_pool(name="pp", bufs=4, space="PSUM") as pp_pool,
            tc.tile_pool(name="ptp", bufs=2, space="PSUM") as ptp_pool,
        ):
            for x_name, w_name, b_name in (
                ("xv", "wv", "bv"),
                ("xq", "wq", "bq"),
                ("xk", "wk", "bk"),
            ):
                is_v = x_name == "xv"
                is_q = x_name == "xq"
                x_dram = t[x_name]
                # 1) inputs arrive pre-transposed [HID, S] from the host.
                # XT is held as [128, 512] half-tiles: the first-half tiles
                # release after m=3, so the next matrix's loads overlap this
                # matrix's second half (smaller PE stall at boundaries).
                xt_dt = PV_DT if is_v else MM_DT
                xt_half = [[None, None] for _ in range(KT)]
                for half in range(2):
                    for k in range(KT):
                        xt_k = xt_pool.tile([P, 512], xt_dt, name=f"xt_{x_name}_{k}_{half}", tag="xt")
                        nc.sync.dma_start(
                            out=xt_k[:],
                            in_=x_dram[k * P:(k + 1) * P, half * 512:(half + 1) * 512],
                        )
                        xt_half[k][half] = xt_k

                # 2) projection: out[s, d] accumulated over hid; d-halves of 512.
                # m is the OUTER loop (weights resident) so each psum finishes
                # early and its eviction overlaps the next m's accumulation --
                # keeps the PE stream dense (HAM stays warm).
                for dh in range(2):
                    dsl = slice(dh * 512, (dh + 1) * 512)
                    wts = []
                    for k in range(KT):
                        wt = w_pool.tile([P, 512], PV_DT if is_v else MM_DT, name=f"w_{x_name}_{dh}_{k}", tag="wst")
                        nc.sync.dma_start(
                            out=wt[:], in_=t[w_name][k * P:(k + 1) * P, dsl]
                        )
                        wts.append(wt)
                    first_block = x_name == "xv" and dh == 0
                    ps = []
                    if first_block:
                        # k-outer over m-quads: first matmul only needs one
                        # (xt, w) tile pair, so the PE starts ~25 us earlier.
                        for mb in (range(0, 4), range(4, 8)):
                            pblk = {}
                            for m in mb:
                                pblk[m] = pp_pool.tile([P, 512], F32, name=f"ps_{x_name}_{dh}_{m}", tag="pp")
                            for k in range(KT):
                                for m in mb:
                                    nc.tensor.matmul(
                                        pblk[m][:],
                                        xt_half[k][m // 4][:, (m % 4) * P:(m % 4 + 1) * P],
                                        wts[k][:],
                                        start=(k == 0),
                                        stop=False,
                                    )
                            for m in mb:
                                nc.tensor.matmul(
                                    pblk[m][:],
                                    ones_pv[:] if is_v else ones_row[:],
                                    brow[b_name][:, dsl],
                                    start=False,
                                    stop=True,
                                )
                                ps.append(pblk[m])
                    else:
                        for m in range(NT):
                            ps_m = pp_pool.tile([P, 512], F32, name=f"ps_{x_name}_{dh}_{m}", tag="pp")
                            for k in range(KT):
                                nc.tensor.matmul(
                                    ps_m[:],
                                    xt_half[k][m // 4][:, (m % 4) * P:(m % 4 + 1) * P],
                                    wts[k][:],
                                    start=(k == 0),
                                    stop=False,
                                )
                            # + bias via ones-matmul (K=1)
                            nc.tensor.matmul(
                                ps_m[:],
                                ones_pv[:] if is_v else ones_row[:],
                                brow[b_name][:, dsl],
                                start=False,
                                stop=True,
                            )
                            ps.append(ps_m)
                    if is_v:
                        for m in range(NT):
                            vo = vout_pool.tile([P, 512], PV_DT, name=f"vo_{dh}_{m}", tag="vout")
                            nc.scalar.activation(vo[:], ps[m][:], AF.Copy)
                            nc.scalar.dma_start(
                                out=v_scratch[m * P:(m + 1) * P, dsl], in_=vo[:]
                            )
                    else:
                        # RoPE on 4 heads of this d-half, then transpose into QT/KT
                        cnm = "cosq" if is_q else "cosk"
                        snm = "sinq" if is_q else "sink"
                        dst_list = QT if is_q else KTt
                        for m in range(NT):
                            msl = slice(m * P, (m + 1) * P)
                            # trig tiles differ per s-tile
                            cos_m = trig_pool.tile([P, 4, 64], F32, name=f"cos_{x_name}_{dh}_{m}", tag="trig")
                            sin_m = trig_pool.tile([P, 4, 64], F32, name=f"sin_{x_name}_{dh}_{m}", tag="trig")
                            nc.sync.dma_start(
                                out=cos_m[:],
                                in_=t[cnm][msl, :].rearrange("p (h f) -> p h f", f=64),
                            )
                            nc.sync.dma_start(
                                out=sin_m[:],
                                in_=t[snm][msl, :].rearrange("p (h f) -> p h f", f=64),
                            )
                            pv = ps[m][:].rearrange("p (h d) -> p h d", d=P)
                            pe = pv[:, :, 0:64]
                            po = pv[:, :, 64:128]
                            rot = rot_pool.tile([P, 4, P], F32, name=f"rot_{x_name}_{dh}_{m}", tag="rot")
                            re = rot[:, :, 0:64]
                            ro = rot[:, :, 64:128]
                            t1 = rtmp_pool.tile([P, 4, 64], F32, name=f"t1_{m}", tag="rt")
                            t2 = rtmp_pool.tile([P, 4, 64], F32, name=f"t2_{m}", tag="rt")
                            t3 = rtmp_pool.tile([P, 4, 64], F32, name=f"t3_{m}", tag="rt")
                            t4 = rtmp_pool.tile([P, 4, 64], F32, name=f"t4_{m}", tag="rt")
                            nc.vector.tensor_mul(t1[:], pe, cos_m[:])
                            nc.vector.tensor_mul(t2[:], po, sin_m[:])
                            nc.gpsimd.tensor_sub(re, t1[:], t2[:])
                            nc.vector.tensor_mul(t3[:], pe, sin_m[:])
                            nc.vector.tensor_mul(t4[:], po, cos_m[:])
                            nc.gpsimd.tensor_add(ro, t3[:], t4[:])
                            # transpose rot [128 s, 4 heads x 128 d] into QT/KT
                            tp2 = ptp_pool.tile([P, 512], F32, name=f"tpR_{x_name}_{dh}_{m}", tag="ptp")
                            for hh in range(4):
                                nc.tensor.transpose(
                                    tp2[:, hh * P:(hh + 1) * P],
                                    rot[:, hh, :],
                                    identity,
                                )
                            for hh in range(4):
                                h = dh * 4 + hh
                                nc.scalar.activation(
                                    dst_list[h][:, msl],
                                    tp2[:, hh * P:(hh + 1) * P],
                                    AF.Copy,
                                )

        # ---------------- Phase A: attention per head -----------------
        prev_out = t["prev_out"]
        out_view_prev = t["preva"]

        ctx_pool = ctx.enter_context(tc.tile_pool(name="ctx", bufs=NHC))
        ctxT = [ctx_pool.tile([P, S], PV_DT, name=f"ctxT{h}", tag="ctx") for h in range(NHC)]

        with (
            tc.tile_pool(name="vh", bufs=3) as vh_pool,
            tc.tile_pool(name="addin", bufs=6) as add_pool,
            tc.tile_pool(name="ssb", bufs=3) as ssb_pool,
            tc.tile_pool(name="probs", bufs=2) as probs_pool,
            tc.tile_pool(name="ptbuf", bufs=2 if USE_BF else 1) as pt_pool,
            tc.tile_pool(name="sums", bufs=3) as sum_pool,
            tc.tile_pool(name="spsum", bufs=3, space="PSUM") as spsum_pool,
            tc.tile_pool(name="tpsum2", bufs=2, space="PSUM") as tpsum2_pool,
        ):
            vs_view = v_scratch[:].rearrange("(m p) d -> p m d", p=P)
            for h in range(NHC):
                hsl = slice(h * P, (h + 1) * P)
                # stream this head's V back in: [128 s, (8 m? no...)]
                # v_h[p, j, :] = v_scratch[j*128 + p, h*128:(h+1)*128]
                v_h = vh_pool.tile([P, NT, P], PV_DT, name=f"vh_{h}", tag="vh")
                nc.sync.dma_start(out=v_h[:], in_=vs_view[:, :, hsl])

                probsT = pt_pool.tile([P, NT, NT, P], PV_DT, name=f"probsT_{h}", tag="ptbuf")
                for m in range(NT):
                    msl = slice(m * P, (m + 1) * P)
                    ps_s = spsum_pool.tile([P, S], F32, name=f"ps_s_{h}_{m}", tag="spsum")
                    at = add_pool.tile([P, S], MM_DT, name=f"prev_{h}_{m}", tag="adda")
                    if os.environ.get("KATQ", "sync") == "gpsimd":
                        nc.gpsimd.dma_start(out=at[:], in_=out_view_prev[h, msl, :])
                    else:
                        nc.sync.dma_start(out=at[:], in_=out_view_prev[h, msl, :])
                    for nh in range(2):
                        nsl = slice(nh * 512, (nh + 1) * 512)
                        nc.tensor.matmul(
                            ps_s[:, nsl],
                            QT[h][:, msl],
                            KTt[h][:, nsl],
                            start=True,
                            stop=False,
                        )
                        # + (bias/sqrt(D) + prev_attn + mask) via identity matmul
                        nc.tensor.matmul(
                            ps_s[:, nsl], identity_mm[:], at[:, nsl],
                            start=False, stop=True,
                        )
                    # scores -> prev_out (exact linear output)
                    s_sb = ssb_pool.tile([P, S], F32, name=f"s_sb_{h}_{m}", tag="ssb")
                    nc.vector.tensor_copy(s_sb[:], ps_s[:])
                    nc.scalar.dma_start(out=prev_out[h, msl, :], in_=s_sb[:])
                    # softmax (no max-subtraction: |scores| is O(10) here)
                    probs = probs_pool.tile([P, S], PV_DT, name=f"probs_{h}_{m}", tag="probs")
                    sums = sum_pool.tile([P, 2], F32, name=f"sums_{h}_{m}", tag="sums")
                    nc.scalar.activation(
                        probs[:], ps_s[:], AF.Exp, accum_out=sums[:, 0:1]
                    )
                    nc.vector.reciprocal(sums[:, 1:2], sums[:, 0:1])
                    nc.vector.tensor_scalar_mul(probs[:], probs[:], sums[:, 1:2])
                    # transpose probs into probsT[p, m, j, :]
                    for g in range(2):
                        tp = tpsum2_pool.tile([P, 512], PV_DT, name=f"tpP_{h}_{m}_{g}", tag="tp2")
                        for jj in range(4):
                            j = g * 4 + jj
                            nc.tensor.transpose(
                                tp[:, jj * P:(jj + 1) * P],
                                probs[:, j * P:(j + 1) * P],
                                identity_pv,
                            )
                        if g == 0:
                            nc.scalar.activation(
                                probsT[:, m, 0:4, :], tp[:], AF.Copy
                            )
                        else:
                            nc.vector.tensor_copy(probsT[:, m, 4:8, :], tp[:])
                # PV: ctxT_h[d, q] = sum_j V[j,h].T @ probsT[j]
                for g in range(2):
                    gsl = slice(g * 512, (g + 1) * 512)
                    ps_c = tpsum2_pool.tile([P, 512], F32, name=f"ps_c_{h}_{g}", tag="tp2")
                    for j in range(NT):
                        nc.tensor.matmul(
                            ps_c[:],
                            v_h[:, j, :],
                            probsT[:, g * 4:(g + 1) * 4, j, :],
                            start=(j == 0),
                            stop=(j == NT - 1),
                        )
                    nc.vector.tensor_copy(ctxT[h][:, gsl], ps_c[:])



        # ---------------- Phase O: output projection -----------------
        with (
            tc.tile_pool(name="wo", bufs=NHC) as wo_pool,
            tc.tile_pool(name="osb", bufs=2) as osb_pool,
            tc.tile_pool(name="opsum", bufs=4, space="PSUM") as opsum_pool,
        ):
            wo_sb = []
            for h in range(NHC):
                wt = wo_pool.tile([P, HID], PV_DT, name=f"wo_{h}", tag="wo")
                nc.sync.dma_start(out=wt[:], in_=t["wo"][h * P:(h + 1) * P, :])
                wo_sb.append(wt)
            for m in range(NT):
                msl = slice(m * P, (m + 1) * P)
                o_sb = osb_pool.tile([P, HID], F32, name=f"osb_{m}", tag="osb")
                for n in range(4):
                    nsl = slice(n * 512, (n + 1) * 512)
                    ps_o = opsum_pool.tile([P, 512], F32, name=f"ps_o_{m}_{n}", tag="opsum")
                    for h in range(NHC):
                        nc.tensor.matmul(
                            ps_o[:],
                            ctxT[h][:, msl],
                            wo_sb[h][:, nsl],
                            start=(h == 0),
                            stop=(h == NHC - 1),
                        )
                    nc.vector.tensor_copy(o_sb[:, nsl], ps_o[:])
                nc.scalar.dma_start(out=t["out_p"][msl, :], in_=o_sb[:])


def build_program():
    nc = bacc.Bacc("TRN2", target_bir_lowering=False, debug=False)
    t = {}
    t["xq"] = nc.dram_tensor("xq", [HID, S], MM_DT, kind="ExternalInput")
    t["xk"] = nc.dram_tensor("xk", [HID, S], MM_DT, kind="ExternalInput")
    t["xv"] = nc.dram_tensor("xv", [HID, S], PV_DT, kind="ExternalInput")
    t["wq"] = nc.dram_tensor("wq", [HID, NHC * D], MM_DT, kind="ExternalInput")
    t["wk"] = nc.dram_tensor("wk", [HID, NHC * D], MM_DT, kind="ExternalInput")
    t["wv"] = nc.dram_tensor("wv", [HID, NHC * D], PV_DT, kind="ExternalInput")
    t["wo"] = nc.dram_tensor("wo", [NHC * D, HID], PV_DT, kind="ExternalInput")
    t["bq"] = nc.dram_tensor("bq", [1, NHC * D], MM_DT, kind="ExternalInput")
    t["bk"] = nc.dram_tensor("bk", [1, NHC * D], MM_DT, kind="ExternalInput")
    t["bv"] = nc.dram_tensor("bv", [1, NHC * D], PV_DT, kind="ExternalInput")
    t["cosq"] = nc.dram_tensor("cosq", [S, 256], F32, kind="ExternalInput")
    t["sinq"] = nc.dram_tensor("sinq", [S, 256], F32, kind="ExternalInput")
    t["cosk"] = nc.dram_tensor("cosk", [S, 256], F32, kind="ExternalInput")
    t["sink"] = nc.dram_tensor("sink", [S, 256], F32, kind="ExternalInput")
    t["preva"] = nc.dram_tensor("preva", [NHC, S, S], MM_DT, kind="ExternalInput")
    t["prev_out"] = nc.dram_tensor("prev_out", [NHC, S, S], F32, kind="ExternalOutput")
    t["out_p"] = nc.dram_tensor("out_p", [S, HID], F32, kind="ExternalOutput")
    t["v_scratch"] = nc.dram_tensor("v_scratch", [S, NHC * D], PV_DT, kind="Internal")
    t["ones_c"] = nc.dram_tensor("ones_c", [1, P], MM_DT, kind="ExternalInput")
    t["ident_c"] = nc.dram_tensor("ident_c", [P, P], MM_DT, kind="ExternalInput")

    with tile.TileContext(nc) as tc:
        emit(nc, tc, t)
    nc.compile()
    return nc


_NC_CACHE = None


def _get_program():
    global _NC_CACHE
    if _NC_CACHE is None:
        _NC_CACHE = build_program()
    return _NC_CACHE


def make_in_maps(inputs):
    """Host-side sharding + layout prep. Returns list of 8 per-core input maps."""
    f = lambda x: np.ascontiguousarray(np.asarray(x, dtype=np.float32))
    query, key, value = f(inputs["query"]), f(inputs["key"]), f(inputs["value"])
    mask = f(inputs["mask"])            # [B,1,S,S]
    bias = f(inputs["position_bias"])   # [1,NH,S,S]
    prev = f(inputs["prev_attn"])       # [B,NH,S,S]
    freqs = f(inputs["freqs_cis"])      # [S,64,2]
    wq, wk, wv, wo = f(inputs["wq"]), f(inputs["wk"]), f(inputs["wv"]), f(inputs["wo"])
    bq, bk, bv = f(inputs["bq"]), f(inputs["bk"]), f(inputs["bv"])

    # de-interleave RoPE pairs within each head: d' = [0,2,...,126, 1,3,...,127]
    perm = np.concatenate([np.arange(0, D, 2), np.arange(1, D, 2)])
    wq_p = wq.reshape(HID, NH, D)[:, :, perm]
    wk_p = wk.reshape(HID, NH, D)[:, :, perm]
    bq_p = bq.reshape(NH, D)[:, perm]
    bk_p = bk.reshape(NH, D)[:, perm]

    cos = freqs[:, :, 0]  # [S, 64]
    sin = freqs[:, :, 1]
    cos_q = np.ascontiguousarray(np.tile(cos / SQD, (1, 4)))  # [S, 256]
    sin_q = np.ascontiguousarray(np.tile(sin / SQD, (1, 4)))
    cos_k = np.ascontiguousarray(np.tile(cos, (1, 4)))
    sin_k = np.ascontiguousarray(np.tile(sin, (1, 4)))

    # combined additive term: bias/sqrt(D) (shared over batch) + prev + mask
    biasp = bias[0] / SQD                       # [NH,S,S]
    preva = prev + mask + biasp[None]           # [B,NH,S,S]

    wo_r = wo.reshape(NH, D, HID)

    queryT = np.ascontiguousarray(query.transpose(0, 2, 1))  # [B, HID, S]
    keyT = np.ascontiguousarray(key.transpose(0, 2, 1))
    valueT = np.ascontiguousarray(value.transpose(0, 2, 1))

    in_maps = []
    for c in range(8):
        b, g = divmod(c, 2)
        hs = slice(g * NHC, (g + 1) * NHC)
        in_maps.append({
            "xq": queryT[b],
            "xk": keyT[b],
            "xv": _pv_cast(valueT[b]),
            "wq": np.ascontiguousarray(wq_p[:, hs].reshape(HID, NHC * D)),
            "wk": np.ascontiguousarray(wk_p[:, hs].reshape(HID, NHC * D)),
            "wv": _pv_cast(np.ascontiguousarray(wv[:, g * NHC * D:(g + 1) * NHC * D])),
            "wo": _pv_cast(np.ascontiguousarray(wo_r[hs].reshape(NHC * D, HID))),
            "bq": np.ascontiguousarray(bq_p[hs].reshape(1, NHC * D)),
            "bk": np.ascontiguousarray(bk_p[hs].reshape(1, NHC * D)),
            "bv": _pv_cast(np.ascontiguousarray(bv[g * NHC * D:(g + 1) * NHC * D].reshape(1, NHC * D))),
            "cosq": cos_q,
            "sinq": sin_q,
            "cosk": cos_k,
            "sink": sin_k,
            "preva": np.ascontiguousarray(preva[b, hs]),
            "ones_c": np.ones((1, D), dtype=np.float32),
            "ident_c": np.eye(D, dtype=np.float32),
        })
    return in_maps


def gather_outputs(results, inputs):
    bo = np.asarray(inputs["bo"], dtype=np.float32)
    out = np.zeros((B, S, HID), dtype=np.float32)
    prev_out = np.empty((B, NH, S, S), dtype=np.float32)
    for c in range(8):
        b, g = divmod(c, 2)
        hs = slice(g * NHC, (g + 1) * NHC)
        out[b] += results[c]["out_p"]
        prev_out[b, hs] = results[c]["prev_out"]
    out += bo[None, None, :]
    return out, prev_out


def kernel(**inputs):
    from concourse.bass_utils import run_bass_kernel_spmd

    nc = _get_program()
    in_maps = make_in_maps(inputs)
    res = run_bass_kernel_spmd(nc, in_maps, list(range(8)))
    return gather_outputs(res.results, inputs)


# revision 29
# speedup vs baseline: 1.0332x; 1.0332x over previous
"""Multi-headed attention (RoPE + position_bias + residual attention) on 8 TRN2 cores.

Contract: kernel(**inputs) takes the FULL inputs of reference.setup_inputs()
and returns (out, prev_attn_out) as the reference does.

Sharding: 8 cores = 4 batches x 2 head-groups (8 heads each).
  - Each core computes its batch's QKV projections for its 8 heads,
    RoPE, scores (+bias/sqrt(D) + prev_attn + mask), softmax, PV, and a
    partial output projection (row-parallel over heads).
  - Host sums the two partial projections per batch and adds bo.

All device compute is emitted once as a single SPMD Bass/Tile program; the
per-core differences are entirely in the input tensors.
"""

import math
import os

import ml_dtypes
import numpy as np

def _pv_cast(a):
    return a.astype(ml_dtypes.bfloat16) if os.environ.get("KBF", "0") == "1" else a

import concourse.bacc as bacc
import concourse.bass as bass
import concourse.mybir as mybir
import concourse.tile as tile
from concourse.masks import make_identity

F32 = mybir.dt.float32
BF16 = mybir.dt.bfloat16
AF = mybir.ActivationFunctionType

B, S, HID, NH, D = 4, 1024, 2048, 16, 128
NHC = 8            # heads per core
P = 128            # partitions
NT = S // P        # 8 s-tiles
KT = HID // P      # 16 hid k-tiles
SQD = math.sqrt(float(D))

# matmul compute dtype: float32r = fp32 data, reduced-precision (fast) PE path.
# walrus requires every producer of an fp32r-matmul operand to emit float32r,
# so the operand tiles (and the DRAM tensors DMA'd straight into them) are
# declared float32r rather than bitcast at use.
MM_DT = mybir.dt.float32r if os.environ.get("KMM", "f32r") == "f32r" else F32
# optional bf16 on the probs/v/out chains (adds ~4e-3 rel err on `out`)
USE_BF = os.environ.get("KBF", "0") == "1"
PV_DT = BF16 if USE_BF else MM_DT


def emit(nc, tc, t):
    """Emit the per-core program. `t` maps tensor names -> DRAM handles."""
    import contextlib

    ctx = contextlib.ExitStack()
    with ctx:
        const = ctx.enter_context(tc.tile_pool(name="const", bufs=1))
        identity = const.tile([P, P], F32, name="identity")
        make_identity(nc, identity)
        # f32r constants cannot be produced by memset/affine_select
        # (walrus rejects the ISA) so they arrive as tiny DMA'd inputs.
        ones_row = const.tile([1, P], MM_DT, name="ones_row")
        nc.sync.dma_start(out=ones_row[:], in_=t["ones_c"][:])
        if USE_BF:
            identity_pv = const.tile([P, P], BF16, name="identity_pv")
            make_identity(nc, identity_pv)
            ones_pv = const.tile([1, P], BF16, name="ones_pv")
            nc.vector.memset(ones_pv[:], 1.0)
        else:
            identity_pv = const.tile([P, P], MM_DT, name="identity_pv")
            nc.sync.dma_start(out=identity_pv[:], in_=t["ident_c"][:])
            ones_pv = ones_row
        # identity in matmul dtype for the PE-side additive-term matmuls
        identity_mm = const.tile([P, P], MM_DT, name="identity_mm")
        nc.sync.dma_start(out=identity_mm[:], in_=t["ident_c"][:])

        # biases as [1, 1024] rows in sbuf
        brow = {}
        for nm in ("bq", "bk", "bv"):
            bdt = PV_DT if nm == "bv" else MM_DT
            brow[nm] = const.tile([1, NHC * D], bdt, name=f"{nm}_sb")
            nc.sync.dma_start(out=brow[nm][:], in_=t[nm][:])

        # persistent results of the projection phase
        qkt_pool = ctx.enter_context(tc.tile_pool(name="qkt", bufs=2 * NHC))
        QT = [qkt_pool.tile([P, S], MM_DT, name=f"QT{h}", tag="qkt") for h in range(NHC)]
        KTt = [qkt_pool.tile([P, S], MM_DT, name=f"KT{h}", tag="qkt") for h in range(NHC)]

        v_scratch = t["v_scratch"]

        # ---------------- Phase T+P: transposes + projections -----------------
        with (
            tc.tile_pool(name="xt", bufs=38) as xt_pool,
            tc.tile_pool(name="wst", bufs=KT) as w_pool,
            tc.tile_pool(name="trig", bufs=2) as trig_pool,
            tc.tile_pool(name="ropetmp", bufs=6) as rtmp_pool,
            tc.tile_pool(name="rotout", bufs=2) as rot_pool,
            tc.tile_pool(name="vout", bufs=2) as vout_pool,
            tc.tile_pool(name="pp", bufs=4, space="PSUM") as pp_pool,
            tc.tile_pool(name="ptp", bufs=2, space="PSUM") as ptp_pool,
        ):
            for x_name, w_name, b_name in (
                ("xv", "wv", "bv"),
                ("xq", "wq", "bq"),
                ("xk", "wk", "bk"),
            ):
                is_v = x_name == "xv"
                is_q = x_name == "xq"
                x_dram = t[x_name]
                # 1) inputs arrive pre-transposed [HID, S] from the host.
                # XT is held as [128, 512] half-tiles: the first-half tiles
                # release after m=3, so the next matrix's loads overlap this
                # matrix's second half (smaller PE stall at boundaries).
                xt_dt = PV_DT if is_v else MM_DT
                xt_half = [[None, None] for _ in range(KT)]
                for half in range(2):
                    for k in range(KT):
                        xt_k = xt_pool.tile([P, 512], xt_dt, name=f"xt_{x_name}_{k}_{half}", tag="xt")
                        nc.sync.dma_start(
                            out=xt_k[:],
                            in_=x_dram[k * P:(k + 1) * P, half * 512:(half + 1) * 512],
                        )
                        xt_half[k][half] = xt_k

                # 2) projection: out[s, d] accumulated over hid; d-halves of 512.
                # m is the OUTER loop (weights resident) so each psum finishes
                # early and its eviction overlaps the next m's accumulation --
                # keeps the PE stream dense (HAM stays warm).
                for dh in range(2):
                    dsl = slice(dh * 512, (dh + 1) * 512)
                    wts = []
                    for k in range(KT):
                        wt = w_pool.tile([P, 512], PV_DT if is_v else MM_DT, name=f"w_{x_name}_{dh}_{k}", tag="wst")
                        nc.sync.dma_start(
                            out=wt[:], in_=t[w_name][k * P:(k + 1) * P, dsl]
                        )
                        wts.append(wt)
                    first_block = x_name == "xv" and dh == 0
                    ps = []
                    if first_block:
                        # k-outer over m-quads: first matmul only needs one
                        # (xt, w) tile pair, so the PE starts ~25 us earlier.
                        for mb in (range(0, 4), range(4, 8)):
                            pblk = {}
                            for m in mb:
                                pblk[m] = pp_pool.tile([P, 512], F32, name=f"ps_{x_name}_{dh}_{m}", tag="pp")
                            for k in range(KT):
                                for m in mb:
                                    nc.tensor.matmul(
                                        pblk[m][:],
                                        xt_half[k][m // 4][:, (m % 4) * P:(m % 4 + 1) * P],
                                        wts[k][:],
                                        start=(k == 0),
                                        stop=False,
                                    )
                            for m in mb:
                                nc.tensor.matmul(
                                    pblk[m][:],
                                    ones_pv[:] if is_v else ones_row[:],
                                    brow[b_name][:, dsl],
                                    start=False,
                                    stop=True,
                                )
                                ps.append(pblk[m])
                    else:
                        for m in range(NT):
                            ps_m = pp_pool.tile([P, 512], F32, name=f"ps_{x_name}_{dh}_{m}", tag="pp")
                            for k in range(KT):
                                nc.tensor.matmul(
                                    ps_m[:],
                                    xt_half[k][m // 4][:, (m % 4) * P:(m % 4 + 1) * P],
                                    wts[k][:],
                                    start=(k == 0),
                                    stop=False,
                                )
                            # + bias via ones-matmul (K=1)
                            nc.tensor.matmul(
                                ps_m[:],
                                ones_pv[:] if is_v else ones_row[:],
                                brow[b_name][:, dsl],
                                start=False,
                                stop=True,
                            )
                            ps.append(ps_m)
                    if is_v:
                        for m in range(NT):
                            vo = vout_pool.tile([P, 512], PV_DT, name=f"vo_{dh}_{m}", tag="vout")
                            nc.scalar.activation(vo[:], ps[m][:], AF.Copy)
                            nc.scalar.dma_start(
                                out=v_scratch[m * P:(m + 1) * P, dsl], in_=vo[:]
                            )
                    else:
                        # RoPE on 4 heads of this d-half, then transpose into QT/KT
                        cnm = "cosq" if is_q else "cosk"
                        snm = "sinq" if is_q else "sink"
                        dst_list = QT if is_q else KTt
                        for m in range(NT):
                            msl = slice(m * P, (m + 1) * P)
                            # trig tiles differ per s-tile
                            cos_m = trig_pool.tile([P, 4, 64], F32, name=f"cos_{x_name}_{dh}_{m}", tag="trig")
                            sin_m = trig_pool.tile([P, 4, 64], F32, name=f"sin_{x_name}_{dh}_{m}", tag="trig")
                            nc.sync.dma_start(
                                out=cos_m[:],
                                in_=t[cnm][msl, :].rearrange("p (h f) -> p h f", f=64),
                            )
                            nc.sync.dma_start(
                                out=sin_m[:],
                                in_=t[snm][msl, :].rearrange("p (h f) -> p h f", f=64),
                            )
                            pv = ps[m][:].rearrange("p (h d) -> p h d", d=P)
                            pe = pv[:, :, 0:64]
                            po = pv[:, :, 64:128]
                            rot = rot_pool.tile([P, 4, P], F32, name=f"rot_{x_name}_{dh}_{m}", tag="rot")
                            re = rot[:, :, 0:64]
                            ro = rot[:, :, 64:128]
                            t1 = rtmp_pool.tile([P, 4, 64], F32, name=f"t1_{m}", tag="rt")
                            t2 = rtmp_pool.tile([P, 4, 64], F32, name=f"t2_{m}", tag="rt")
                            t3 = rtmp_pool.tile([P, 4, 64], F32, name=f"t3_{m}", tag="rt")
                            t4 = rtmp_pool.tile([P, 4, 64], F32, name=f"t4_{m}", tag="rt")
                            nc.vector.tensor_mul(t1[:], pe, cos_m[:])
                            nc.vector.tensor_mul(t2[:], po, sin_m[:])
                            nc.gpsimd.tensor_sub(re, t1[:], t2[:])
                            nc.vector.tensor_mul(t3[:], pe, sin_m[:])
                            nc.vector.tensor_mul(t4[:], po, cos_m[:])
                            nc.gpsimd.tensor_add(ro, t3[:], t4[:])
                            # transpose rot [128 s, 4 heads x 128 d] into QT/KT
                            tp2 = ptp_pool.tile([P, 512], F32, name=f"tpR_{x_name}_{dh}_{m}", tag="ptp")
                            for hh in range(4):
                                nc.tensor.transpose(
                                    tp2[:, hh * P:(hh + 1) * P],
                                    rot[:, hh, :],
                                    identity,
                                )
                            for hh in range(4):
                                h = dh * 4 + hh
                                nc.scalar.activation(
                                    dst_list[h][:, msl],
                                    tp2[:, hh * P:(hh + 1) * P],
                                    AF.Copy,
                                )

        # ---------------- Phase A: attention per head -----------------
        prev_out = t["prev_out"]
        out_view_prev = t["preva"]

        ctx_pool = ctx.enter_context(tc.tile_pool(name="ctx", bufs=NHC))
        ctxT = [ctx_pool.tile([P, S], PV_DT, name=f"ctxT{h}", tag="ctx") for h in range(NHC)]

        with (
            tc.tile_pool(name="vh", bufs=3) as vh_pool,
            tc.tile_pool(name="addin", bufs=6) as add_pool,
            tc.tile_pool(name="ssb", bufs=3) as ssb_pool,
            tc.tile_pool(name="probs", bufs=2) as probs_pool,
            tc.tile_pool(name="ptbuf", bufs=2 if USE_BF else 1) as pt_pool,
            tc.tile_pool(name="sums", bufs=3) as sum_pool,
            tc.tile_pool(name="spsum", bufs=3, space="PSUM") as spsum_pool,
            tc.tile_pool(name="tpsum2", bufs=2, space="PSUM") as tpsum2_pool,
        ):
            vs_view = v_scratch[:].rearrange("(m p) d -> p m d", p=P)
            for h in range(NHC):
                hsl = slice(h * P, (h + 1) * P)
                # stream this head's V back in: [128 s, (8 m? no...)]
                # v_h[p, j, :] = v_scratch[j*128 + p, h*128:(h+1)*128]
                v_h = vh_pool.tile([P, NT, P], PV_DT, name=f"vh_{h}", tag="vh")
                nc.sync.dma_start(out=v_h[:], in_=vs_view[:, :, hsl])

                probsT = pt_pool.tile([P, NT, NT, P], PV_DT, name=f"probsT_{h}", tag="ptbuf")
                for m in range(NT):
                    msl = slice(m * P, (m + 1) * P)
                    ps_s = spsum_pool.tile([P, S], F32, name=f"ps_s_{h}_{m}", tag="spsum")
                    at = add_pool.tile([P, S], MM_DT, name=f"prev_{h}_{m}", tag="adda")
                    if os.environ.get("KATQ", "sync") == "gpsimd":
                        nc.gpsimd.dma_start(out=at[:], in_=out_view_prev[h, msl, :])
                    else:
                        nc.sync.dma_start(out=at[:], in_=out_view_prev[h, msl, :])
                    for nh in range(2):
                        nsl = slice(nh * 512, (nh + 1) * 512)
                        nc.tensor.matmul(
                            ps_s[:, nsl],
                            QT[h][:, msl],
                            KTt[h][:, nsl],
                            start=True,
                            stop=False,
                        )
                        # + (bias/sqrt(D) + prev_attn + mask) via identity matmul
                        nc.tensor.matmul(
                            ps_s[:, nsl], identity_mm[:], at[:, nsl],
                            start=False, stop=True,
                        )
                    # scores -> prev_out (exact linear output); copy split
                    # across ACT and DVE to balance the per-tile chain
                    s_sb = ssb_pool.tile([P, S], F32, name=f"s_sb_{h}_{m}", tag="ssb")
                    nc.scalar.activation(s_sb[:, 0:512], ps_s[:, 0:512], AF.Copy)
                    nc.vector.tensor_copy(s_sb[:, 512:1024], ps_s[:, 512:1024])
                    nc.scalar.dma_start(out=prev_out[h, msl, :], in_=s_sb[:])
                    # softmax (no max-subtraction: |scores| is O(10) here)
                    probs = probs_pool.tile([P, S], PV_DT, name=f"probs_{h}_{m}", tag="probs")
                    sums = sum_pool.tile([P, 2], F32, name=f"sums_{h}_{m}", tag="sums")
                    nc.scalar.activation(
                        probs[:], ps_s[:], AF.Exp, accum_out=sums[:, 0:1]
                    )
                    nc.vector.reciprocal(sums[:, 1:2], sums[:, 0:1])
                    nc.vector.tensor_scalar_mul(probs[:], probs[:], sums[:, 1:2])
                    # transpose probs into probsT[p, m, j, :]
                    for g in range(2):
                        tp = tpsum2_pool.tile([P, 512], PV_DT, name=f"tpP_{h}_{m}_{g}", tag="tp2")
                        for jj in range(4):
                            j = g * 4 + jj
                            nc.tensor.transpose(
                                tp[:, jj * P:(jj + 1) * P],
                                probs[:, j * P:(j + 1) * P],
                                identity_pv,
                            )
                        if g == 0:
                            nc.scalar.activation(
                                probsT[:, m, 0:4, :], tp[:], AF.Copy
                            )
                        else:
                            nc.vector.tensor_copy(probsT[:, m, 4:8, :], tp[:])
                # PV: ctxT_h[d, q] = sum_j V[j,h].T @ probsT[j]
                for g in range(2):
                    gsl = slice(g * 512, (g + 1) * 512)
                    ps_c = tpsum2_pool.tile([P, 512], F32, name=f"ps_c_{h}_{g}", tag="tp2")
                    for j in range(NT):
                        nc.tensor.matmul(
                            ps_c[:],
                            v_h[:, j, :],
                            probsT[:, g * 4:(g + 1) * 4, j, :],
                            start=(j == 0),
                            stop=(j == NT - 1),
                        )
                    nc.vector.tensor_copy(ctxT[h][:, gsl], ps_c[:])



        # ---------------- Phase O: output projection -----------------
        with (
            tc.tile_pool(name="wo", bufs=NHC) as wo_pool,
            tc.tile_pool(name="osb", bufs=2) as osb_pool,
            tc.tile_pool(name="opsum", bufs=4, space="PSUM") as opsum_pool,
        ):
            wo_sb = []
            for h in range(NHC):
                wt = wo_pool.tile([P, HID], PV_DT, name=f"wo_{h}", tag="wo")
                nc.sync.dma_start(out=wt[:], in_=t["wo"][h * P:(h + 1) * P, :])
                wo_sb.append(wt)
            for m in range(NT):
                msl = slice(m * P, (m + 1) * P)
                o_sb = osb_pool.tile([P, HID], F32, name=f"osb_{m}", tag="osb")
                for n in range(4):
                    nsl = slice(n * 512, (n + 1) * 512)
                    ps_o = opsum_pool.tile([P, 512], F32, name=f"ps_o_{m}_{n}", tag="opsum")
                    for h in range(NHC):
                        nc.tensor.matmul(
                            ps_o[:],
                            ctxT[h][:, msl],
                            wo_sb[h][:, nsl],
                            start=(h == 0),
                            stop=(h == NHC - 1),
                        )
                    nc.vector.tensor_copy(o_sb[:, nsl], ps_o[:])
                nc.scalar.dma_start(out=t["out_p"][msl, :], in_=o_sb[:])


def build_program():
    nc = bacc.Bacc("TRN2", target_bir_lowering=False, debug=False)
    t = {}
    t["xq"] = nc.dram_tensor("xq", [HID, S], MM_DT, kind="ExternalInput")
    t["xk"] = nc.dram_tensor("xk", [HID, S], MM_DT, kind="ExternalInput")
    t["xv"] = nc.dram_tensor("xv", [HID, S], PV_DT, kind="ExternalInput")
    t["wq"] = nc.dram_tensor("wq", [HID, NHC * D], MM_DT, kind="ExternalInput")
    t["wk"] = nc.dram_tensor("wk", [HID, NHC * D], MM_DT, kind="ExternalInput")
    t["wv"] = nc.dram_tensor("wv", [HID, NHC * D], PV_DT, kind="ExternalInput")
    t["wo"] = nc.dram_tensor("wo", [NHC * D, HID], PV_DT, kind="ExternalInput")
    t["bq"] = nc.dram_tensor("bq", [1, NHC * D], MM_DT, kind="ExternalInput")
    t["bk"] = nc.dram_tensor("bk", [1, NHC * D], MM_DT, kind="ExternalInput")
    t["bv"] = nc.dram_tensor("bv", [1, NHC * D], PV_DT, kind="ExternalInput")
    t["cosq"] = nc.dram_tensor("cosq", [S, 256], F32, kind="ExternalInput")
    t["sinq"] = nc.dram_tensor("sinq", [S, 256], F32, kind="ExternalInput")
    t["cosk"] = nc.dram_tensor("cosk", [S, 256], F32, kind="ExternalInput")
    t["sink"] = nc.dram_tensor("sink", [S, 256], F32, kind="ExternalInput")
    t["preva"] = nc.dram_tensor("preva", [NHC, S, S], MM_DT, kind="ExternalInput")
    t["prev_out"] = nc.dram_tensor("prev_out", [NHC, S, S], F32, kind="ExternalOutput")
    t["out_p"] = nc.dram_tensor("out_p", [S, HID], F32, kind="ExternalOutput")
    t["v_scratch"] = nc.dram_tensor("v_scratch", [S, NHC * D], PV_DT, kind="Internal")
    t["ones_c"] = nc.dram_tensor("ones_c", [1, P], MM_DT, kind="ExternalInput")
    t["ident_c"] = nc.dram_tensor("ident_c", [P, P], MM_DT, kind="ExternalInput")

    with tile.TileContext(nc) as tc:
        emit(nc, tc, t)
    nc.compile()
    return nc


_NC_CACHE = None


def _get_program():
    global _NC_CACHE
    if _NC_CACHE is None:
        _NC_CACHE = build_program()
    return _NC_CACHE


def make_in_maps(inputs):
    """Host-side sharding + layout prep. Returns list of 8 per-core input maps."""
    f = lambda x: np.ascontiguousarray(np.asarray(x, dtype=np.float32))
    query, key, value = f(inputs["query"]), f(inputs["key"]), f(inputs["value"])
    mask = f(inputs["mask"])            # [B,1,S,S]
    bias = f(inputs["position_bias"])   # [1,NH,S,S]
    prev = f(inputs["prev_attn"])       # [B,NH,S,S]
    freqs = f(inputs["freqs_cis"])      # [S,64,2]
    wq, wk, wv, wo = f(inputs["wq"]), f(inputs["wk"]), f(inputs["wv"]), f(inputs["wo"])
    bq, bk, bv = f(inputs["bq"]), f(inputs["bk"]), f(inputs["bv"])

    # de-interleave RoPE pairs within each head: d' = [0,2,...,126, 1,3,...,127]
    perm = np.concatenate([np.arange(0, D, 2), np.arange(1, D, 2)])
    wq_p = wq.reshape(HID, NH, D)[:, :, perm]
    wk_p = wk.reshape(HID, NH, D)[:, :, perm]
    bq_p = bq.reshape(NH, D)[:, perm]
    bk_p = bk.reshape(NH, D)[:, perm]

    cos = freqs[:, :, 0]  # [S, 64]
    sin = freqs[:, :, 1]
    cos_q = np.ascontiguousarray(np.tile(cos / SQD, (1, 4)))  # [S, 256]
    sin_q = np.ascontiguousarray(np.tile(sin / SQD, (1, 4)))
    cos_k = np.ascontiguousarray(np.tile(cos, (1, 4)))
    sin_k = np.ascontiguousarray(np.tile(sin, (1, 4)))

    # combined additive term: bias/sqrt(D) (shared over batch) + prev + mask
    biasp = bias[0] / SQD                       # [NH,S,S]
    preva = prev + mask + biasp[None]           # [B,NH,S,S]

    wo_r = wo.reshape(NH, D, HID)

    queryT = np.ascontiguousarray(query.transpose(0, 2, 1))  # [B, HID, S]
    keyT = np.ascontiguousarray(key.transpose(0, 2, 1))
    valueT = np.ascontiguousarray(value.transpose(0, 2, 1))

    in_maps = []
    for c in range(8):
        b, g = divmod(c, 2)
        hs = slice(g * NHC, (g + 1) * NHC)
        in_maps.append({
            "xq": queryT[b],
            "xk": keyT[b],
            "xv": _pv_cast(valueT[b]),
            "wq": np.ascontiguousarray(wq_p[:, hs].reshape(HID, NHC * D)),
            "wk": np.ascontiguousarray(wk_p[:, hs].reshape(HID, NHC * D)),
            "wv": _pv_cast(np.ascontiguousarray(wv[:, g * NHC * D:(g + 1) * NHC * D])),
            "wo": _pv_cast(np.ascontiguousarray(wo_r[hs].reshape(NHC * D, HID))),
            "bq": np.ascontiguousarray(bq_p[hs].reshape(1, NHC * D)),
            "bk": np.ascontiguousarray(bk_p[hs].reshape(1, NHC * D)),
            "bv": _pv_cast(np.ascontiguousarray(bv[g * NHC * D:(g + 1) * NHC * D].reshape(1, NHC * D))),
            "cosq": cos_q,
            "sinq": sin_q,
            "cosk": cos_k,
            "sink": sin_k,
            "preva": np.ascontiguousarray(preva[b, hs]),
            "ones_c": np.ones((1, D), dtype=np.float32),
            "ident_c": np.eye(D, dtype=np.float32),
        })
    return in_maps


def gather_outputs(results, inputs):
    bo = np.asarray(inputs["bo"], dtype=np.float32)
    out = np.zeros((B, S, HID), dtype=np.float32)
    prev_out = np.empty((B, NH, S, S), dtype=np.float32)
    for c in range(8):
        b, g = divmod(c, 2)
        hs = slice(g * NHC, (g + 1) * NHC)
        out[b] += results[c]["out_p"]
        prev_out[b, hs] = results[c]["prev_out"]
    out += bo[None, None, :]
    return out, prev_out


def kernel(**inputs):
    from concourse.bass_utils import run_bass_kernel_spmd

    nc = _get_program()
    in_maps = make_in_maps(inputs)
    res = run_bass_kernel_spmd(nc, in_maps, list(range(8)))
    return gather_outputs(res.results, inputs)
